# revision 31
# baseline (speedup 1.0000x reference)
"""Causal self-attention (B=2, T=2048, C=1024, H=16) on 8 trn2 NeuronCores.

Sharding: core = (batch b, head-group g) with 4 heads per group.
  - data parallel over B (2 ways) x tensor parallel over heads (4 ways)
  - each core computes qkv for its head group, causal attention for its
    4 heads, and a partial proj (its 256 rows of w_proj); the host sums
    the 4 per-batch partials (deferred tensor-parallel all-reduce).

Device layout (per core, all fp32 in memory, float32r on the PE):
  xt    [1024, 2048]  x^T for this core's batch (host pre-transposed)
  wqkv  [1024, 768]   [q_g | k_g | v_g] columns of w_qkv for this group
  wproj [256, 1024]   rows of w_proj for this group
  masks [128, 2048]   4 causal staircase masks (diag offsets 0..3 * 128)
  out   [2048, 1024]  partial output (host sums groups)

The attention math is arranged so no on-device transposes are needed:
  q^T,k^T [d, t] come straight out of the qkv matmul (lhsT = w slice,
  rhs = x^T); S^T[tk,tq] = k^T.T @ q^T-moving; exp on ACT; y^T and the
  softmax denominator come from one AV matmul with a ones-column
  appended to v (lhsT = v_aug), and proj consumes y^T directly as lhsT.
Softmax skips max-subtraction (scores ~ N(0,1) after 1/sqrt(D): exp is
safe in fp32), matching the reference up to fp rounding.
"""

import os
import sys
from contextlib import ExitStack

import numpy as np

for _p in ("/opt/trn_rl_repo", "/root/.axon_site/_ro/trn_rl_repo"):
    if os.path.isdir(_p) and _p not in sys.path:
        sys.path.insert(0, _p)

import concourse.bass as bass
import concourse.bacc as bacc
import concourse.mybir as mybir
import concourse.tile as tile
from concourse.tile_rust import add_dep_helper
from concourse.bass_utils import run_bass_kernel_spmd

B, T, C, H, D = 2, 2048, 1024, 16, 64
GH = 4                 # heads per core (group)
GC = GH * D            # 256 channels per group
NCORES = 8
TQ = 512               # query tile (free dim of S^T / AV matmuls)
TK = 128               # key tile (partition dim of S^T)
NB = T // TQ           # 4 query blocks
NT = T // TK           # 16 key tiles
CK = C // 128          # 8 contraction chunks for qkv
F32 = mybir.dt.float32
F32R = mybir.dt.float32r

EXPF = mybir.ActivationFunctionType.Exp

_CACHE = {}


def _r(ap):
    """View an fp32 AP as float32r (TF32; same 4-byte container)."""
    return ap.bitcast(mybir.dt.float32r)


def _build_bass(repeat=1):
    nc = bacc.Bacc("TRN2", target_bir_lowering=False, debug=False)
    xt = nc.declare_dram_parameter("xt", [C, T], F32R, isOutput=False)
    wqkv = nc.declare_dram_parameter("wqkv", [C, 3 * GC], F32R, isOutput=False)
    wproj = nc.declare_dram_parameter("wproj", [GC, C], F32R, isOutput=False)
    masks = nc.declare_dram_parameter("masks", [128, 4 * TQ], F32R, isOutput=False)
    out = nc.declare_dram_parameter("out", [T, C], F32, isOutput=True)

    with ExitStack() as ctx:
        tc = ctx.enter_context(tile.TileContext(nc))
        consts = ctx.enter_context(tc.tile_pool(name="consts", bufs=1))
        persist = ctx.enter_context(tc.tile_pool(name="persist", bufs=1))
        xpool = ctx.enter_context(tc.tile_pool(name="xp", bufs=2))
        espool = ctx.enter_context(tc.tile_pool(name="es", bufs=4))
        rpool = ctx.enter_context(tc.tile_pool(name="rp", bufs=2))
        rbpool = ctx.enter_context(tc.tile_pool(name="rb", bufs=2))
        opool = ctx.enter_context(tc.tile_pool(name="op", bufs=3))
        dpool = ctx.enter_context(tc.tile_pool(name="dp", bufs=2, space="DRAM"))
        # 8 fixed PSUM tiles (one bank each), rotated manually. Pool-based
        # PSUM slot reuse emits unconditional multi-sem release waits on the
        # claiming matmul, which blows the 1-wait budget of f32r self-loading
        # matmuls; with fixed tiles reuse is plain WAR/WAW dep tracking.
        psum = ctx.enter_context(tc.tile_pool(name="psum", bufs=1, space="PSUM"))
        PSD = [
            psum.tile([128, 2, TQ], F32, tag=f"psd{r}", name=f"psd{r}")
            for r in range(2)
        ]
        PP = [psum.tile([128, TQ], F32, tag="pp0", name="pp0")]
        PJ = [psum.tile([128, TQ], F32, tag="pj0", name="pj0")]
        AVD = psum.tile([D + 1, 2, TQ], F32, tag="avd", name="avd")
        cnt = {"pp": 0, "pj": 0, "ps": 0}

        # ---- constants / persistent tiles ----
        # (DMAs for weights/masks are issued inside qkv_block(0), interleaved
        # after the x chunks, so the first matmuls start as early as possible)
        w_sb = consts.tile([128, CK, 3 * GC], F32R, tag="wqkv")
        wqkv_r = wqkv[:].rearrange("(a p) n -> p a n", p=128)
        wp_sb = consts.tile([128, 2, C], F32R, tag="wproj")
        wproj_r = wproj[:].rearrange("(a p) n -> p a n", p=128)
        mk_sb = consts.tile([128, 4 * TQ], F32R, tag="masks")

        qT = [
            persist.tile([128, T], F32R, tag=f"qT{p}", name=f"qT{p}") for p in range(2)
        ]
        kT = [
            persist.tile([128, T], F32R, tag=f"kT{p}", name=f"kT{p}") for p in range(2)
        ]
        yT = [
            persist.tile([128, T], F32R, tag=f"yT{p}", name=f"yT{p}") for p in range(2)
        ]
        vaug = persist.tile([128, NT, GH * (D + 1)], F32R, tag="vaug")

        def load_consts_tail():
            nc.sync.dma_start(out=mk_sb[:], in_=masks[:])
            for k in range(2):
                nc.sync.dma_start(out=wp_sb[:, k, :], in_=wproj_r[:, k, :])
            for h in range(GH):
                # mk_sb[:, TQ-1] is all-ones (d=0 staircase, last column)
                nc.vector.tensor_copy(
                    vaug[:, :, h * (D + 1) + D : (h + 1) * (D + 1)],
                    mk_sb[:, TQ - 1 : TQ].unsqueeze(1).broadcast_to((128, NT, 1)),
                )

        def qkv_block(tb, rep):
            x_sb = xpool.tile([128, CK, TQ], F32R, tag="x")
            xt_r = xt[:, tb * TQ : (tb + 1) * TQ].rearrange("(a p) n -> p a n", p=128)
            first = tb == 0 and rep == 0
            for k in range(CK):
                nc.sync.dma_start(out=x_sb[:, k, :], in_=xt_r[:, k, :])
                if first:
                    nc.sync.dma_start(out=w_sb[:, k, :], in_=wqkv_r[:, k, :])
            if first:
                load_consts_tail()
            # q^T / k^T for both head pairs
            for pair in range(2):
                for which, dest in ((0, qT), (1, kT)):
                    pq = PP[0]
                    cnt["pp"] += 1
                    for k in range(CK):
                        cols = which * GC + pair * 128
                        nc.tensor.matmul(
                            pq[:],
                            w_sb[:, k, cols : cols + 128],
                            x_sb[:, k, :],
                            start=(k == 0),
                            stop=(k == CK - 1),
                        )
                    nc.vector.tensor_copy(
                        dest[pair][:, tb * TQ : (tb + 1) * TQ], _r(pq[:])
                    )
            # v for the 4 key tiles of this block
            for tt in range(TQ // TK):
                jt = tb * (TQ // TK) + tt
                pv = PP[0]
                cnt["pp"] += 1
                for k in range(CK):
                    nc.tensor.matmul(
                        pv[:, 0:GC],
                        x_sb[:, k, tt * TK : (tt + 1) * TK],
                        w_sb[:, k, 2 * GC : 3 * GC],
                        start=(k == 0),
                        stop=(k == CK - 1),
                    )
                nc.vector.tensor_copy(
                    vaug[:, jt, :]
                    .rearrange("p (h c) -> p h c", h=GH)[:, :, 0:D],
                    _r(pv[:, 0:GC].rearrange("p (h c) -> p h c", h=GH)),
                )

        def attn_block(pair, i):
            jmax = (TQ // TK) * (i + 1)
            for j in range(jmax):
                dg = j - (TQ // TK) * i  # >=0 on the diagonal band
                # valid region of this tile is cols [dg*TK, TQ); compute only
                # [c0, TQ) with c0 capped so the f32r moving dim stays >=256
                c0 = min(dg, 2) * TK if dg > 0 else 0
                cs = slice(c0, TQ)
                psd = PSD[cnt["ps"] % 2]
                cnt["ps"] += 1
                for half in range(2):
                    lo, hi = half * D, half * D + D
                    kap = kT[pair][lo:hi, j * TK : (j + 1) * TK]
                    qap = qT[pair][lo:hi, i * TQ + c0 : (i + 1) * TQ]
                    nc.tensor.matmul(
                        psd[:, half, cs], kap, qap, start=True, stop=True
                    )
                est = espool.tile([128, 2, TQ], F32R, tag="es", name="est")
                for half in range(2):
                    nc.scalar.activation(
                        est[:, half, cs], _r(psd[:, half, cs]), EXPF, scale=0.125
                    )
                if dg >= 0:
                    nc.vector.tensor_mul(
                        est[:, :, cs],
                        est[:, :, cs],
                        mk_sb[:, dg * TQ + c0 : (dg + 1) * TQ]
                        .unsqueeze(1)
                        .broadcast_to((128, 2, TQ - c0)),
                    )
                for half in range(2):
                    h = pair * 2 + half
                    vap = vaug[:, j, h * (D + 1) : (h + 1) * (D + 1)]
                    nc.tensor.matmul(
                        AVD[:, half, cs],
                        vap,
                        est[:, half, cs],
                        start=(j == 0),
                        stop=(j == jmax - 1),
                    )
            # normalize: y = y_unnorm / denom (denom in row D of av psum)
            tqs = slice(i * TQ, (i + 1) * TQ)
            avs = rpool.tile([D + 1, 2, TQ], F32, tag="avs", name="avs")
            for half in range(2):
                nc.vector.tensor_copy(avs[:, half, :], AVD[:, half, :])
            # reciprocal lands in a f32r tile (row D, lane-aligned) so the
            # broadcast matmul's input is produced as rounded f32r
            r32 = rpool.tile([D + 1, 2, TQ], F32R, tag="r32", name="r32")
            with nc.allow_low_precision(reason="TF32 softmax denominators"):
                nc.vector.reciprocal(r32[D : D + 1, :, :], avs[D : D + 1, :, :])
            # broadcast 1/denom across partitions with a rank-1 PE matmul
            # into the just-freed AVD banks (lhsT = ones row at partition 64,
            # taken from the mask; rhs = the reciprocal row)
            ones_row = mk_sb[D : D + 1, 2 * TK : 2 * TK + D]
            for half in range(2):
                nc.tensor.matmul(
                    AVD[0:D, half, :],
                    ones_row,
                    r32[D : D + 1, half, :],
                    start=True,
                    stop=True,
                )
            nc.vector.tensor_mul(
                yT[pair][0:D, tqs], _r(avs[0:D, 0, :]), _r(AVD[0:D, 0, :])
            )
            nc.vector.tensor_mul(
                avs[0:D, 1, :], avs[0:D, 1, :], AVD[0:D, 1, :]
            )
            nc.sync.dma_start(
                out=yT[pair][D : 2 * D, tqs], in_=_r(avs[0:D, 1, :])
            )

        def proj_block(i):
            for tt in range(TQ // TK):
                tq0 = i * TQ + tt * TK
                osb = opool.tile([128, C], F32, tag="o")
                for half in range(2):
                    po = PJ[0]
                    cnt["pj"] += 1
                    for pair in range(2):
                        yap = yT[pair][:, tq0 : tq0 + TK]
                        wap = wp_sb[:, pair, half * 512 : (half + 1) * 512]
                        nc.tensor.matmul(
                            po[:],
                            yap,
                            wap,
                            start=(pair == 0),
                            stop=(pair == 1),
                        )
                    nc.vector.tensor_copy(
                        osb[:, half * 512 : (half + 1) * 512], _r(po[:])
                    )
                    nc.sync.dma_start(
                        out=out[tq0 : tq0 + TK, half * 512 : (half + 1) * 512],
                        in_=osb[:, half * 512 : (half + 1) * 512],
                    )

        for _rep in range(repeat):
            for tb in range(NB):
                qkv_block(tb, _rep)
                for pair in range(2):
                    attn_block(pair, tb)
                proj_block(tb)

    nc.compile()
    return nc


def _host_shards(x, w_qkv, w_proj):
    x = np.asarray(x, dtype=np.float32)
    w_qkv = np.asarray(w_qkv, dtype=np.float32)
    w_proj = np.asarray(w_proj, dtype=np.float32)

    p = np.arange(128)[:, None]
    c = np.arange(TQ)[None, :]
    masks = np.concatenate(
        [(p + d * TK <= c).astype(np.float32) for d in range(4)], axis=1
    )  # [128, 2048]

    in_maps = []
    for core in range(NCORES):
        b, g = divmod(core, NCORES // B)
        qc = w_qkv[:, g * GC : (g + 1) * GC]
        kc = w_qkv[:, C + g * GC : C + (g + 1) * GC]
        vc = w_qkv[:, 2 * C + g * GC : 2 * C + (g + 1) * GC]
        in_maps.append(
            {
                "xt": np.ascontiguousarray(x[b].T),
                "wqkv": np.ascontiguousarray(np.concatenate([qc, kc, vc], axis=1)),
                "wproj": np.ascontiguousarray(w_proj[g * GC : (g + 1) * GC, :]),
                "masks": masks,
            }
        )
    return in_maps


def kernel(x, w_qkv, w_proj, _trace=False, _trace_kwargs=None):
    if "nc" not in _CACHE:
        _CACHE["nc"] = _build_bass()
    nc = _CACHE["nc"]
    in_maps = _host_shards(x, w_qkv, w_proj)
    res = run_bass_kernel_spmd(
        nc,
        in_maps,
        core_ids=list(range(NCORES)),
        trace=_trace,
        **(_trace_kwargs or {}),
    )
    _CACHE["last_result"] = res
    g_per_b = NCORES // B
    out = np.stack(
        [
            np.sum([res.results[b * g_per_b + g]["out"] for g in range(g_per_b)], axis=0)
            for b in range(B)
        ]
    ).astype(np.float32)
    return out


# revision 35
# speedup vs baseline: 25.3614x; 25.3614x over previous
"""Causal self-attention (B=2, T=2048, C=1024, H=16) on 8 trn2 NeuronCores.

Sharding: core = (batch b, head-group g) with 4 heads per group.
  - data parallel over B (2 ways) x tensor parallel over heads (4 ways)
  - each core computes qkv for its head group, causal attention for its
    4 heads, and a partial proj (its 256 rows of w_proj); the host sums
    the 4 per-batch partials (deferred tensor-parallel all-reduce).

Device layout (per core, all fp32 in memory, float32r on the PE):
  xt    [1024, 2048]  x^T for this core's batch (host pre-transposed)
  wqkv  [1024, 768]   [q_g | k_g | v_g] columns of w_qkv for this group
  wproj [256, 1024]   rows of w_proj for this group
  masks [128, 2048]   4 causal staircase masks (diag offsets 0..3 * 128)
  out   [2048, 1024]  partial output (host sums groups)

The attention math is arranged so no on-device transposes are needed:
  q^T,k^T [d, t] come straight out of the qkv matmul (lhsT = w slice,
  rhs = x^T); S^T[tk,tq] = k^T.T @ q^T-moving; exp on ACT; y^T and the
  softmax denominator come from one AV matmul with a ones-column
  appended to v (lhsT = v_aug), and proj consumes y^T directly as lhsT.
Softmax skips max-subtraction (scores ~ N(0,1) after 1/sqrt(D): exp is
safe in fp32), matching the reference up to fp rounding.
"""

import os
import sys
from contextlib import ExitStack

import numpy as np

for _p in ("/opt/trn_rl_repo", "/root/.axon_site/_ro/trn_rl_repo"):
    if os.path.isdir(_p) and _p not in sys.path:
        sys.path.insert(0, _p)

import concourse.bass as bass
import concourse.bacc as bacc
import concourse.mybir as mybir
import concourse.tile as tile
from concourse.tile_rust import add_dep_helper
from concourse.bass_utils import run_bass_kernel_spmd

B, T, C, H, D = 2, 2048, 1024, 16, 64
GH = 4                 # heads per core (group)
GC = GH * D            # 256 channels per group
NCORES = 8
TQ = 512               # query tile (free dim of S^T / AV matmuls)
TK = 128               # key tile (partition dim of S^T)
NB = T // TQ           # 4 query blocks
NT = T // TK           # 16 key tiles
CK = C // 128          # 8 contraction chunks for qkv
F32 = mybir.dt.float32
F32R = mybir.dt.float32r

EXPF = mybir.ActivationFunctionType.Exp

_CACHE = {}


def _r(ap):
    """View an fp32 AP as float32r (TF32; same 4-byte container)."""
    return ap.bitcast(mybir.dt.float32r)


def _build_bass(repeat=1):
    nc = bacc.Bacc("TRN2", target_bir_lowering=False, debug=False)
    xt = nc.declare_dram_parameter("xt", [C, T], F32R, isOutput=False)
    wqkv = nc.declare_dram_parameter("wqkv", [C, 3 * GC], F32R, isOutput=False)
    wproj = nc.declare_dram_parameter("wproj", [GC, C], F32R, isOutput=False)
    masks = nc.declare_dram_parameter("masks", [128, 4 * TQ], F32R, isOutput=False)
    out = nc.declare_dram_parameter("out", [T, C], F32, isOutput=True)

    with ExitStack() as ctx:
        tc = ctx.enter_context(tile.TileContext(nc))
        consts = ctx.enter_context(tc.tile_pool(name="consts", bufs=1))
        persist = ctx.enter_context(tc.tile_pool(name="persist", bufs=1))
        xpool = ctx.enter_context(tc.tile_pool(name="xp", bufs=2))
        espool = ctx.enter_context(tc.tile_pool(name="es", bufs=4))
        rpool = ctx.enter_context(tc.tile_pool(name="rp", bufs=2))
        rbpool = ctx.enter_context(tc.tile_pool(name="rb", bufs=2))
        opool = ctx.enter_context(tc.tile_pool(name="op", bufs=3))
        dpool = ctx.enter_context(tc.tile_pool(name="dp", bufs=2, space="DRAM"))
        # 8 fixed PSUM tiles (one bank each), rotated manually. Pool-based
        # PSUM slot reuse emits unconditional multi-sem release waits on the
        # claiming matmul, which blows the 1-wait budget of f32r self-loading
        # matmuls; with fixed tiles reuse is plain WAR/WAW dep tracking.
        psum = ctx.enter_context(tc.tile_pool(name="psum", bufs=1, space="PSUM"))
        PSD = [
            psum.tile([128, 2, TQ], F32, tag=f"psd{r}", name=f"psd{r}")
            for r in range(2)
        ]
        PP = [psum.tile([128, TQ], F32, tag="pp0", name="pp0")]
        PJ = [psum.tile([128, TQ], F32, tag="pj0", name="pj0")]
        AVD = psum.tile([D + 1, 2, TQ], F32, tag="avd", name="avd")
        cnt = {"pp": 0, "pj": 0, "ps": 0}

        # ---- constants / persistent tiles ----
        # (DMAs for weights/masks are issued inside qkv_block(0), interleaved
        # after the x chunks, so the first matmuls start as early as possible)
        w_sb = consts.tile([128, CK, 3 * GC], F32R, tag="wqkv")
        wqkv_r = wqkv[:].rearrange("(a p) n -> p a n", p=128)
        wp_sb = consts.tile([128, 2, C], F32R, tag="wproj")
        wproj_r = wproj[:].rearrange("(a p) n -> p a n", p=128)
        mk_sb = consts.tile([128, 4 * TQ], F32R, tag="masks")

        qT = [
            persist.tile([128, T], F32R, tag=f"qT{p}", name=f"qT{p}") for p in range(2)
        ]
        kT = [
            persist.tile([128, T], F32R, tag=f"kT{p}", name=f"kT{p}") for p in range(2)
        ]
        yT = [
            persist.tile([128, T], F32R, tag=f"yT{p}", name=f"yT{p}") for p in range(2)
        ]
        vaug = persist.tile([128, NT, GH * (D + 1)], F32R, tag="vaug")

        def load_consts_tail():
            nc.sync.dma_start(out=mk_sb[:], in_=masks[:])
            for k in range(2):
                nc.sync.dma_start(out=wp_sb[:, k, :], in_=wproj_r[:, k, :])
            for h in range(GH):
                # mk_sb[:, TQ-1] is all-ones (d=0 staircase, last column)
                nc.vector.tensor_copy(
                    vaug[:, :, h * (D + 1) + D : (h + 1) * (D + 1)],
                    mk_sb[:, TQ - 1 : TQ].unsqueeze(1).broadcast_to((128, NT, 1)),
                )

        def qkv_block(tb, rep):
            x_sb = xpool.tile([128, CK, TQ], F32R, tag="x")
            xt_r = xt[:, tb * TQ : (tb + 1) * TQ].rearrange("(a p) n -> p a n", p=128)
            first = tb == 0 and rep == 0
            for k in range(CK):
                nc.sync.dma_start(out=x_sb[:, k, :], in_=xt_r[:, k, :])
                if first:
                    nc.sync.dma_start(out=w_sb[:, k, :], in_=wqkv_r[:, k, :])
            if first:
                load_consts_tail()
            # q^T / k^T for both head pairs
            for pair in range(2):
                for which, dest in ((0, qT), (1, kT)):
                    pq = PP[0]
                    cnt["pp"] += 1
                    for k in range(CK):
                        cols = which * GC + pair * 128
                        nc.tensor.matmul(
                            pq[:],
                            w_sb[:, k, cols : cols + 128],
                            x_sb[:, k, :],
                            start=(k == 0),
                            stop=(k == CK - 1),
                        )
                    nc.vector.tensor_copy(
                        dest[pair][:, tb * TQ : (tb + 1) * TQ], _r(pq[:])
                    )
            # v for the 4 key tiles of this block
            for tt in range(TQ // TK):
                jt = tb * (TQ // TK) + tt
                pv = PP[0]
                cnt["pp"] += 1
                for k in range(CK):
                    nc.tensor.matmul(
                        pv[:, 0:GC],
                        x_sb[:, k, tt * TK : (tt + 1) * TK],
                        w_sb[:, k, 2 * GC : 3 * GC],
                        start=(k == 0),
                        stop=(k == CK - 1),
                    )
                nc.vector.tensor_copy(
                    vaug[:, jt, :]
                    .rearrange("p (h c) -> p h c", h=GH)[:, :, 0:D],
                    _r(pv[:, 0:GC].rearrange("p (h c) -> p h c", h=GH)),
                )

        def attn_block(pair, i, last=False):
            jmax = (TQ // TK) * (i + 1)
            for j in range(jmax):
                dg = j - (TQ // TK) * i  # >=0 on the diagonal band
                # valid region of this tile is cols [dg*TK, TQ); compute only
                # [c0, TQ) with c0 capped so the f32r moving dim stays >=256
                c0 = min(dg, 2) * TK if dg > 0 else 0
                cs = slice(c0, TQ)
                psd = PSD[cnt["ps"] % 2]
                cnt["ps"] += 1
                for half in range(2):
                    lo, hi = half * D, half * D + D
                    kap = kT[pair][lo:hi, j * TK : (j + 1) * TK]
                    qap = qT[pair][lo:hi, i * TQ + c0 : (i + 1) * TQ]
                    nc.tensor.matmul(
                        psd[:, half, cs], kap, qap, start=True, stop=True
                    )
                est = espool.tile([128, 2, TQ], F32R, tag="es", name="est")
                for half in range(2):
                    nc.scalar.activation(
                        est[:, half, cs], _r(psd[:, half, cs]), EXPF, scale=0.125
                    )
                if dg >= 0:
                    nc.vector.tensor_mul(
                        est[:, :, cs],
                        est[:, :, cs],
                        mk_sb[:, dg * TQ + c0 : (dg + 1) * TQ]
                        .unsqueeze(1)
                        .broadcast_to((128, 2, TQ - c0)),
                    )
                for half in range(2):
                    h = pair * 2 + half
                    vap = vaug[:, j, h * (D + 1) : (h + 1) * (D + 1)]
                    nc.tensor.matmul(
                        AVD[:, half, cs],
                        vap,
                        est[:, half, cs],
                        start=(j == 0),
                        stop=(j == jmax - 1),
                    )
            # normalize: y = y_unnorm / denom (denom in row D of av psum)
            tqs = slice(i * TQ, (i + 1) * TQ)
            avs = rpool.tile([D + 1, 2, TQ], F32, tag="avs", name="avs")
            for half in range(2):
                nc.vector.tensor_copy(avs[:, half, :], AVD[:, half, :])
            # reciprocal lands in a f32r tile (row D, lane-aligned)
            r32 = rpool.tile([D + 1, 2, TQ], F32R, tag="r32", name="r32")
            with nc.allow_low_precision(reason="TF32 softmax denominators"):
                nc.vector.reciprocal(r32[D : D + 1, :, :], avs[D : D + 1, :, :])
            if last:
                # broadcast 1/denom with a rank-1 PE matmul into the AVD
                # banks: lowest latency, but holds AVD (fine on the last
                # block where no further attention needs it)
                ones_row = mk_sb[D : D + 1, 2 * TK : 2 * TK + D]
                for half in range(2):
                    nc.tensor.matmul(
                        AVD[0:D, half, :],
                        ones_row,
                        r32[D : D + 1, half, :],
                        start=True,
                        stop=True,
                    )
                rbA = AVD[0:D, 0, :]
                rbB = AVD[0:D, 1, :]
            else:
                # DRAM-bounce broadcast: higher latency but AVD stays free
                # so the next attention block's AV can start immediately
                scr = dpool.tile([2, TQ], F32R, tag="scr", name="scr")
                nc.sync.dma_start(out=scr[:], in_=r32[D : D + 1, :, :])
                rbt = [
                    rbpool.tile([D, TQ], F32R, tag=f"rb{h}", name=f"rb{h}")
                    for h in range(2)
                ]
                nc.sync.dma_start(
                    out=rbt[0][:], in_=scr[0:1, :].to_broadcast((D, TQ))
                )
                nc.sync.dma_start(
                    out=rbt[1][:], in_=scr[1:2, :].to_broadcast((D, TQ))
                )
                rbA = rbt[0][:]
                rbB = rbt[1][:]
            nc.vector.tensor_mul(
                yT[pair][0:D, tqs], _r(avs[0:D, 0, :]), rbA
            )
            nc.vector.tensor_mul(avs[0:D, 1, :], avs[0:D, 1, :], rbB)
            nc.sync.dma_start(
                out=yT[pair][D : 2 * D, tqs], in_=_r(avs[0:D, 1, :])
            )

        def proj_block(i):
            for tt in range(TQ // TK):
                tq0 = i * TQ + tt * TK
                osb = opool.tile([128, C], F32, tag="o")
                for half in range(2):
                    po = PJ[0]
                    cnt["pj"] += 1
                    for pair in range(2):
                        yap = yT[pair][:, tq0 : tq0 + TK]
                        wap = wp_sb[:, pair, half * 512 : (half + 1) * 512]
                        nc.tensor.matmul(
                            po[:],
                            yap,
                            wap,
                            start=(pair == 0),
                            stop=(pair == 1),
                        )
                    nc.vector.tensor_copy(
                        osb[:, half * 512 : (half + 1) * 512], _r(po[:])
                    )
                    nc.sync.dma_start(
                        out=out[tq0 : tq0 + TK, half * 512 : (half + 1) * 512],
                        in_=osb[:, half * 512 : (half + 1) * 512],
                    )

        for _rep in range(repeat):
            for tb in range(NB):
                qkv_block(tb, _rep)
                for pair in range(2):
                    attn_block(pair, tb, last=(tb == NB - 1))
                proj_block(tb)

    nc.compile()
    return nc


def _host_shards(x, w_qkv, w_proj):
    x = np.asarray(x, dtype=np.float32)
    w_qkv = np.asarray(w_qkv, dtype=np.float32)
    w_proj = np.asarray(w_proj, dtype=np.float32)

    p = np.arange(128)[:, None]
    c = np.arange(TQ)[None, :]
    masks = np.concatenate(
        [(p + d * TK <= c).astype(np.float32) for d in range(4)], axis=1
    )  # [128, 2048]

    in_maps = []
    for core in range(NCORES):
        b, g = divmod(core, NCORES // B)
        qc = w_qkv[:, g * GC : (g + 1) * GC]
        kc = w_qkv[:, C + g * GC : C + (g + 1) * GC]
        vc = w_qkv[:, 2 * C + g * GC : 2 * C + (g + 1) * GC]
        in_maps.append(
            {
                "xt": np.ascontiguousarray(x[b].T),
                "wqkv": np.ascontiguousarray(np.concatenate([qc, kc, vc], axis=1)),
                "wproj": np.ascontiguousarray(w_proj[g * GC : (g + 1) * GC, :]),
                "masks": masks,
            }
        )
    return in_maps


def kernel(x, w_qkv, w_proj, _trace=False, _trace_kwargs=None):
    if "nc" not in _CACHE:
        _CACHE["nc"] = _build_bass()
    nc = _CACHE["nc"]
    in_maps = _host_shards(x, w_qkv, w_proj)
    res = run_bass_kernel_spmd(
        nc,
        in_maps,
        core_ids=list(range(NCORES)),
        trace=_trace,
        **(_trace_kwargs or {}),
    )
    _CACHE["last_result"] = res
    g_per_b = NCORES // B
    out = np.stack(
        [
            np.sum([res.results[b * g_per_b + g]["out"] for g in range(g_per_b)], axis=0)
            for b in range(B)
        ]
    ).astype(np.float32)
    return out


# revision 36
# speedup vs baseline: 26.5893x; 1.0484x over previous
"""Causal self-attention (B=2, T=2048, C=1024, H=16) on 8 trn2 NeuronCores.

Sharding: core = (batch b, head-group g) with 4 heads per group.
  - data parallel over B (2 ways) x tensor parallel over heads (4 ways)
  - each core computes qkv for its head group, causal attention for its
    4 heads, and a partial proj (its 256 rows of w_proj); the host sums
    the 4 per-batch partials (deferred tensor-parallel all-reduce).

Device layout (per core, all fp32 in memory, float32r on the PE):
  xt    [1024, 2048]  x^T for this core's batch (host pre-transposed)
  wqkv  [1024, 768]   [q_g | k_g | v_g] columns of w_qkv for this group
  wproj [256, 1024]   rows of w_proj for this group
  masks [128, 2048]   4 causal staircase masks (diag offsets 0..3 * 128)
  out   [2048, 1024]  partial output (host sums groups)

The attention math is arranged so no on-device transposes are needed:
  q^T,k^T [d, t] come straight out of the qkv matmul (lhsT = w slice,
  rhs = x^T); S^T[tk,tq] = k^T.T @ q^T-moving; exp on ACT; y^T and the
  softmax denominator come from one AV matmul with a ones-column
  appended to v (lhsT = v_aug), and proj consumes y^T directly as lhsT.
Softmax skips max-subtraction (scores ~ N(0,1) after 1/sqrt(D): exp is
safe in fp32), matching the reference up to fp rounding.
"""

import os
import sys
from contextlib import ExitStack

import numpy as np

for _p in ("/opt/trn_rl_repo", "/root/.axon_site/_ro/trn_rl_repo"):
    if os.path.isdir(_p) and _p not in sys.path:
        sys.path.insert(0, _p)

import concourse.bass as bass
import concourse.bacc as bacc
import concourse.mybir as mybir
import concourse.tile as tile
from concourse.bass_utils import run_bass_kernel_spmd

B, T, C, H, D = 2, 2048, 1024, 16, 64
GH = 4                 # heads per core (group)
GC = GH * D            # 256 channels per group
NCORES = 8
TQ = 512               # query tile (free dim of S^T / AV matmuls)
TK = 128               # key tile (partition dim of S^T)
NB = T // TQ           # 4 query blocks
NT = T // TK           # 16 key tiles
CK = C // 128          # 8 contraction chunks for qkv
F32 = mybir.dt.float32
F32R = mybir.dt.float32r

EXPF = mybir.ActivationFunctionType.Exp

_CACHE = {}


def _r(ap):
    """View an fp32 AP as float32r (TF32; same 4-byte container)."""
    return ap.bitcast(mybir.dt.float32r)


def _build_bass(repeat=1):
    nc = bacc.Bacc("TRN2", target_bir_lowering=False, debug=False)
    xt = nc.declare_dram_parameter("xt", [C, T], F32R, isOutput=False)
    wqkv = nc.declare_dram_parameter("wqkv", [C, 3 * GC], F32R, isOutput=False)
    wproj = nc.declare_dram_parameter("wproj", [GC, C], F32R, isOutput=False)
    masks = nc.declare_dram_parameter("masks", [128, 4 * TQ], F32R, isOutput=False)
    out = nc.declare_dram_parameter("out", [T, C], F32, isOutput=True)

    with ExitStack() as ctx:
        tc = ctx.enter_context(tile.TileContext(nc))
        consts = ctx.enter_context(tc.tile_pool(name="consts", bufs=1))
        persist = ctx.enter_context(tc.tile_pool(name="persist", bufs=1))
        xpool = ctx.enter_context(tc.tile_pool(name="xp", bufs=2))
        espool = ctx.enter_context(tc.tile_pool(name="es", bufs=4))
        rpool = ctx.enter_context(tc.tile_pool(name="rp", bufs=2))
        rbpool = ctx.enter_context(tc.tile_pool(name="rb", bufs=2))
        opool = ctx.enter_context(tc.tile_pool(name="op", bufs=3))
        dpool = ctx.enter_context(tc.tile_pool(name="dp", bufs=2, space="DRAM"))
        # 8 fixed PSUM tiles (one bank each), rotated manually. Pool-based
        # PSUM slot reuse emits unconditional multi-sem release waits on the
        # claiming matmul, which blows the 1-wait budget of f32r self-loading
        # matmuls; with fixed tiles reuse is plain WAR/WAW dep tracking.
        psum = ctx.enter_context(tc.tile_pool(name="psum", bufs=1, space="PSUM"))
        PSD = [
            psum.tile([128, 2, TQ], F32, tag=f"psd{r}", name=f"psd{r}")
            for r in range(2)
        ]
        PP = [psum.tile([128, TQ], F32, tag="pp0", name="pp0")]
        PJ = [psum.tile([128, TQ], F32, tag="pj0", name="pj0")]
        AVD = psum.tile([D + 1, 2, TQ], F32, tag="avd", name="avd")
        cnt = {"pp": 0, "pj": 0, "ps": 0}

        # ---- constants / persistent tiles ----
        # (DMAs for weights/masks are issued inside qkv_block(0), interleaved
        # after the x chunks, so the first matmuls start as early as possible)
        w_sb = consts.tile([128, CK, 3 * GC], F32R, tag="wqkv")
        wqkv_r = wqkv[:].rearrange("(a p) n -> p a n", p=128)
        wp_sb = consts.tile([128, 2, C], F32R, tag="wproj")
        wproj_r = wproj[:].rearrange("(a p) n -> p a n", p=128)
        mk_sb = consts.tile([128, 4 * TQ], F32R, tag="masks")

        qT = [
            persist.tile([128, T], F32R, tag=f"qT{p}", name=f"qT{p}") for p in range(2)
        ]
        kT = [
            persist.tile([128, T], F32R, tag=f"kT{p}", name=f"kT{p}") for p in range(2)
        ]
        yT = [
            persist.tile([128, T], F32R, tag=f"yT{p}", name=f"yT{p}") for p in range(2)
        ]
        vaug = persist.tile([128, NT, GH * (D + 1)], F32R, tag="vaug")

        def load_consts_tail():
            nc.sync.dma_start(out=mk_sb[:], in_=masks[:])
            for k in range(2):
                nc.sync.dma_start(out=wp_sb[:, k, :], in_=wproj_r[:, k, :])
            for h in range(GH):
                # mk_sb[:, TQ-1] is all-ones (d=0 staircase, last column)
                nc.vector.tensor_copy(
                    vaug[:, :, h * (D + 1) + D : (h + 1) * (D + 1)],
                    mk_sb[:, TQ - 1 : TQ].unsqueeze(1).broadcast_to((128, NT, 1)),
                )

        def qkv_block(tb, rep):
            x_sb = xpool.tile([128, CK, TQ], F32R, tag="x")
            xt_r = xt[:, tb * TQ : (tb + 1) * TQ].rearrange("(a p) n -> p a n", p=128)
            first = tb == 0 and rep == 0
            for k in range(CK):
                nc.sync.dma_start(out=x_sb[:, k, :], in_=xt_r[:, k, :])
                if first:
                    nc.sync.dma_start(out=w_sb[:, k, :], in_=wqkv_r[:, k, :])
            if first:
                load_consts_tail()
            # q^T / k^T for both head pairs
            for pair in range(2):
                for which, dest in ((0, qT), (1, kT)):
                    pq = PP[0]
                    cnt["pp"] += 1
                    for k in range(CK):
                        cols = which * GC + pair * 128
                        nc.tensor.matmul(
                            pq[:],
                            w_sb[:, k, cols : cols + 128],
                            x_sb[:, k, :],
                            start=(k == 0),
                            stop=(k == CK - 1),
                        )
                    nc.vector.tensor_copy(
                        dest[pair][:, tb * TQ : (tb + 1) * TQ], _r(pq[:])
                    )
            # v for the 4 key tiles of this block
            for tt in range(TQ // TK):
                jt = tb * (TQ // TK) + tt
                pv = PP[0]
                cnt["pp"] += 1
                for k in range(CK):
                    nc.tensor.matmul(
                        pv[:, 0:GC],
                        x_sb[:, k, tt * TK : (tt + 1) * TK],
                        w_sb[:, k, 2 * GC : 3 * GC],
                        start=(k == 0),
                        stop=(k == CK - 1),
                    )
                nc.vector.tensor_copy(
                    vaug[:, jt, :]
                    .rearrange("p (h c) -> p h c", h=GH)[:, :, 0:D],
                    _r(pv[:, 0:GC].rearrange("p (h c) -> p h c", h=GH)),
                )

        def attn_block(pair, i, last=False):
            jmax = (TQ // TK) * (i + 1)
            for j in range(jmax):
                dg = j - (TQ // TK) * i  # >=0 on the diagonal band
                # valid region of this tile is cols [dg*TK, TQ); compute only
                # [c0, TQ) with c0 capped so the f32r moving dim stays >=256
                c0 = min(dg, 2) * TK if dg > 0 else 0
                cs = slice(c0, TQ)
                psd = PSD[cnt["ps"] % 2]
                cnt["ps"] += 1
                for half in range(2):
                    lo, hi = half * D, half * D + D
                    kap = kT[pair][lo:hi, j * TK : (j + 1) * TK]
                    qap = qT[pair][lo:hi, i * TQ + c0 : (i + 1) * TQ]
                    nc.tensor.matmul(
                        psd[:, half, cs], kap, qap, start=True, stop=True
                    )
                est = espool.tile([128, 2, TQ], F32R, tag="es", name="est")
                for half in range(2):
                    nc.scalar.activation(
                        est[:, half, cs], _r(psd[:, half, cs]), EXPF, scale=0.125
                    )
                if dg >= 0:
                    nc.vector.tensor_mul(
                        est[:, :, cs],
                        est[:, :, cs],
                        mk_sb[:, dg * TQ + c0 : (dg + 1) * TQ]
                        .unsqueeze(1)
                        .broadcast_to((128, 2, TQ - c0)),
                    )
                for half in range(2):
                    h = pair * 2 + half
                    vap = vaug[:, j, h * (D + 1) : (h + 1) * (D + 1)]
                    nc.tensor.matmul(
                        AVD[:, half, cs],
                        vap,
                        est[:, half, cs],
                        start=(j == 0),
                        stop=(j == jmax - 1),
                    )
            # normalize: y = y_unnorm / denom (denom in row D of av psum)
            tqs = slice(i * TQ, (i + 1) * TQ)
            avs = rpool.tile([D + 1, 2, TQ], F32, tag="avs", name="avs")
            for half in range(2):
                nc.vector.tensor_copy(avs[:, half, :], AVD[:, half, :])
            # reciprocal lands in a f32r tile (row D, lane-aligned)
            r32 = rpool.tile([D + 1, 2, TQ], F32R, tag="r32", name="r32")
            with nc.allow_low_precision(reason="TF32 softmax denominators"):
                nc.vector.reciprocal(r32[D : D + 1, :, :], avs[D : D + 1, :, :])
            if last:
                # broadcast 1/denom with a rank-1 PE matmul into the AVD
                # banks: lowest latency, but holds AVD (fine on the last
                # block where no further attention needs it)
                ones_row = mk_sb[D : D + 1, 2 * TK : 2 * TK + D]
                for half in range(2):
                    nc.tensor.matmul(
                        AVD[0:D, half, :],
                        ones_row,
                        r32[D : D + 1, half, :],
                        start=True,
                        stop=True,
                    )
                rbA = AVD[0:D, 0, :]
                rbB = AVD[0:D, 1, :]
            else:
                # DRAM-bounce broadcast: higher latency but AVD stays free
                # so the next attention block's AV can start immediately
                scr = dpool.tile([2, TQ], F32R, tag="scr", name="scr")
                nc.sync.dma_start(out=scr[:], in_=r32[D : D + 1, :, :])
                rbt = [
                    rbpool.tile([D, TQ], F32R, tag=f"rb{h}", name=f"rb{h}")
                    for h in range(2)
                ]
                nc.sync.dma_start(
                    out=rbt[0][:], in_=scr[0:1, :].to_broadcast((D, TQ))
                )
                nc.sync.dma_start(
                    out=rbt[1][:], in_=scr[1:2, :].to_broadcast((D, TQ))
                )
                rbA = rbt[0][:]
                rbB = rbt[1][:]
            nc.vector.tensor_mul(
                yT[pair][0:D, tqs], _r(avs[0:D, 0, :]), rbA
            )
            nc.vector.tensor_mul(avs[0:D, 1, :], avs[0:D, 1, :], rbB)
            nc.sync.dma_start(
                out=yT[pair][D : 2 * D, tqs], in_=_r(avs[0:D, 1, :])
            )

        def proj_block(i):
            for tt in range(TQ // TK):
                tq0 = i * TQ + tt * TK
                osb = opool.tile([128, C], F32, tag="o")
                for half in range(2):
                    po = PJ[0]
                    cnt["pj"] += 1
                    for pair in range(2):
                        yap = yT[pair][:, tq0 : tq0 + TK]
                        wap = wp_sb[:, pair, half * 512 : (half + 1) * 512]
                        nc.tensor.matmul(
                            po[:],
                            yap,
                            wap,
                            start=(pair == 0),
                            stop=(pair == 1),
                        )
                    nc.vector.tensor_copy(
                        osb[:, half * 512 : (half + 1) * 512], _r(po[:])
                    )
                    nc.sync.dma_start(
                        out=out[tq0 : tq0 + TK, half * 512 : (half + 1) * 512],
                        in_=osb[:, half * 512 : (half + 1) * 512],
                    )

        for _rep in range(repeat):
            for tb in range(NB):
                qkv_block(tb, _rep)
                for pair in range(2):
                    attn_block(pair, tb, last=(tb == NB - 1))
                proj_block(tb)

    nc.compile()
    return nc


def _host_shards(x, w_qkv, w_proj):
    x = np.asarray(x, dtype=np.float32)
    w_qkv = np.asarray(w_qkv, dtype=np.float32)
    w_proj = np.asarray(w_proj, dtype=np.float32)

    p = np.arange(128)[:, None]
    c = np.arange(TQ)[None, :]
    masks = np.concatenate(
        [(p + d * TK <= c).astype(np.float32) for d in range(4)], axis=1
    )  # [128, 2048]

    in_maps = []
    for core in range(NCORES):
        b, g = divmod(core, NCORES // B)
        qc = w_qkv[:, g * GC : (g + 1) * GC]
        kc = w_qkv[:, C + g * GC : C + (g + 1) * GC]
        vc = w_qkv[:, 2 * C + g * GC : 2 * C + (g + 1) * GC]
        in_maps.append(
            {
                "xt": np.ascontiguousarray(x[b].T),
                "wqkv": np.ascontiguousarray(np.concatenate([qc, kc, vc], axis=1)),
                "wproj": np.ascontiguousarray(w_proj[g * GC : (g + 1) * GC, :]),
                "masks": masks,
            }
        )
    return in_maps


def kernel(x, w_qkv, w_proj, _trace=False, _trace_kwargs=None):
    if "nc" not in _CACHE:
        _CACHE["nc"] = _build_bass()
    nc = _CACHE["nc"]
    in_maps = _host_shards(x, w_qkv, w_proj)
    res = run_bass_kernel_spmd(
        nc,
        in_maps,
        core_ids=list(range(NCORES)),
        trace=_trace,
        **(_trace_kwargs or {}),
    )
    _CACHE["last_result"] = res
    g_per_b = NCORES // B
    out = np.stack(
        [
            np.sum([res.results[b * g_per_b + g]["out"] for g in range(g_per_b)], axis=0)
            for b in range(B)
        ]
    ).astype(np.float32)
    return out
